# revision 6
# baseline (speedup 1.0000x reference)
"""MultiHeadedAttention Trainium2 kernel (rebalanced sharding).

Problem: B=2, T=2048, D=1024, H=16 heads (DK=64), fp32 in/out, padding mask
on keys. out = softmax(mask(QWq (KWk)^T / 8)) @ (VWv) @ Wo^T + biases.

Sharding (8 cores): core c owns heads (2c, 2c+1) of BOTH batches.  Pair 0 =
those heads applied to batch 0, pair 1 = batch 1.  Fully-masked key chunks
(128 keys) are skipped per batch, so the 25% key padding of batch 1 cuts
scores/exp/attnV work 12.5% on EVERY core instead of idling half the cores.
Each core emits bf16 partial out-projections for both batches; the host sums
the 8 partials per batch (+ bo).

Device pipeline per core (structure inherited from the tuned baseline):
  - activations arrive TRANSPOSED bf16 per batch: xT [1024, 2048] so the
    D-contraction sits on partitions for every projection matmul.
  - k/q projections produce kT/qT [128, pair, 2048] (2 heads x 64 dk on
    partitions); key chunks of batch 1 are packed to active chunks only.
  - attention units (q-half x pair) are software-pipelined: unit u's
    scores/exp interleave with unit u-1's V matmuls; the v-projection rides
    in unit 0; qh=0's output projection rides in units 2-3; qh=1's rides the
    tail halves.
  - exp on ScalarE from PSUM with fused scale (1/8) and per-partition mask
    bias (-30000 on padded keys), bf16 out.
  - attn@V transposed: out2[dk+1, q] with a ones column -> row 64 is the
    softmax denominator for free; normalize via reciprocal + gpsimd
    partition_broadcast + DVE multiply -> xh [128, pair, 1024] bf16.
  - out_partial[tok, :] = xh-chunk (stationary, contraction 128) x Wo-rows
    (moving), bf16 out DMA.
"""

import numpy as np
import ml_dtypes

import concourse.bass as bass
import concourse.bacc as bacc
import concourse.tile as tile
from concourse import mybir
from concourse.bass_utils import run_bass_kernel_spmd

B, T, D, H = 2, 2048, 1024, 16
DK = D // H   # 64
NCORES = 8
KC = T // 128     # 16 key chunks per batch
DCH = D // 128    # 8 contraction chunks
F32 = mybir.dt.float32
BF16 = mybir.dt.bfloat16

MASK_NEG = -30000.0


def _runs(active):
    """Coalesce sorted chunk indices into (start_chunk, n_chunks) runs."""
    runs = []
    for kc in active:
        if runs and runs[-1][0] + runs[-1][1] == kc:
            runs[-1][1] += 1
        else:
            runs.append([kc, 1])
    return [(s, n) for s, n in runs]


def _spans(total, step):
    """Split [0, total) into spans of <= step."""
    return [(t0, min(step, total - t0)) for t0 in range(0, total, step)]


def build_program(active_a, active_b, with_bv: bool):
    na, nb = len(active_a), len(active_b)
    nkc = (na, nb)
    ta, tb = 128 * na, 128 * nb   # packed key-token counts
    nc = bacc.Bacc("TRN2")

    # ---- DRAM parameters (per-core shapes) ----
    xq_d = nc.declare_dram_parameter("xq", [B, DCH, 128, T], BF16, isOutput=False)
    xk_d = nc.declare_dram_parameter("xk", [B, DCH, 128, T], BF16, isOutput=False)
    xv_d = nc.declare_dram_parameter("xv", [B, DCH, 128, T], BF16, isOutput=False)
    wq_d = nc.declare_dram_parameter("wq", [128, DCH, 128], BF16, isOutput=False)
    wk_d = nc.declare_dram_parameter("wk", [128, DCH, 128], BF16, isOutput=False)
    wv_d = nc.declare_dram_parameter("wv", [128, DCH, 128], BF16, isOutput=False)
    wo_d = nc.declare_dram_parameter("wo", [128, D], BF16, isOutput=False)
    mask_d = nc.declare_dram_parameter("maskb", [128, B, KC], F32, isOutput=False)
    bq_d = nc.declare_dram_parameter("bq", [128, 1], F32, isOutput=False)
    bk_d = nc.declare_dram_parameter("bk", [128, 1], F32, isOutput=False)
    bv_d = nc.declare_dram_parameter("bv", [64, 2], F32, isOutput=False)
    out_d = nc.declare_dram_parameter("out", [B, T, D], BF16, isOutput=True)

    kruns = (_runs(active_a), _runs(active_b))

    with tile.TileContext(nc) as tc:
        with (
            tc.tile_pool(name="persist", bufs=1) as pp,
            tc.tile_pool(name="psum", bufs=4, space="PSUM") as psp,
        ):
            wq_sb = pp.tile([128, DCH, 128], BF16, tag="wq")
            wk_sb = pp.tile([128, DCH, 128], BF16, tag="wk")
            wv_sb = pp.tile([128, DCH, 128], BF16, tag="wv")
            wo_sb = pp.tile([128, D], BF16, tag="wo")
            mask_sb = pp.tile([128, B, KC], F32, tag="mask")
            bq_sb = pp.tile([128, 1], F32, tag="bq")
            bk_sb = pp.tile([128, 1], F32, tag="bk")
            bv_sb = pp.tile([64, 2], F32, tag="bv")
            qT_sb = pp.tile([128, 2, T], BF16, tag="qT")
            kT_sb = pp.tile([128, 2, T], BF16, tag="kT")

            xvp_cm = tc.tile_pool(name="xv", bufs=1)
            xvp = xvp_cm.__enter__()
            xv_sb = [[xvp.tile([128, (ta, tb)[p]], BF16, tag=f"xv{p}{k}",
                               name=f"xv{p}{k}") for k in range(DCH)]
                     for p in range(2)]

            # ---- input DMAs + k/q projections ----
            with tc.tile_pool(name="xqk", bufs=1) as xp:
                xq_sb = [[xp.tile([128, T], BF16, tag=f"xq{p}{k}",
                                  name=f"xq{p}{k}") for k in range(DCH)]
                         for p in range(2)]
                xk_sb = [[xp.tile([128, (ta, tb)[p]], BF16, tag=f"xk{p}{k}",
                                  name=f"xk{p}{k}") for k in range(DCH)]
                         for p in range(2)]

                def dma_packed(dst, src_k, p):
                    """DMA active key chunks of batch p, packed contiguously."""
                    off = 0
                    for s, n in kruns[p]:
                        nc.sync.dma_start(
                            out=dst[:, off:off + 128 * n],
                            in_=src_k[:, 128 * s:128 * (s + n)])
                        off += 128 * n

                for k in range(DCH):
                    nc.sync.dma_start(out=wk_sb[:, k:k + 1, :],
                                      in_=wk_d[:, k:k + 1, :])
                    dma_packed(xk_sb[0][k], xk_d[0, k], 0)
                    dma_packed(xk_sb[1][k], xk_d[1, k], 1)
                nc.sync.dma_start(out=bk_sb[:], in_=bk_d[:])
                nc.sync.dma_start(out=bq_sb[:], in_=bq_d[:])
                for k in range(DCH):
                    nc.sync.dma_start(out=wq_sb[:, k:k + 1, :],
                                      in_=wq_d[:, k:k + 1, :])
                    nc.sync.dma_start(out=xq_sb[0][k][:], in_=xq_d[0, k])
                    nc.sync.dma_start(out=xq_sb[1][k][:], in_=xq_d[1, k])
                nc.sync.dma_start(out=mask_sb[:], in_=mask_d[:])
                nc.sync.dma_start(out=wv_sb[:], in_=wv_d[:])
                for k in range(DCH):
                    dma_packed(xv_sb[0][k], xv_d[0, k], 0)
                    dma_packed(xv_sb[1][k], xv_d[1, k], 1)
                nc.sync.dma_start(out=bv_sb[:], in_=bv_d[:])
                nc.sync.dma_start(out=wo_sb[:], in_=wo_d[:])

                # k-outer projection: groups of <=1024 tokens accumulate in
                # parallel psums so the first matmul only waits on chunk 0.
                for w_sb, x_pair, dst, b_sb, toks in (
                    (wk_sb, xk_sb, kT_sb, bk_sb, (ta, tb)),
                    (wq_sb, xq_sb, qT_sb, bq_sb, (T, T)),
                ):
                    groups = [(p, t0, tl) for p in range(2)
                              for t0, tl in _spans(toks[p], 1024)]
                    pst = [psp.tile([128, 1024], F32, tag="ps", name="pst")
                           for _ in groups]
                    for k in range(DCH):
                        for g, (p, t0, tl) in enumerate(groups):
                            for n0, nl in _spans(tl, 512):
                                nc.tensor.matmul(
                                    pst[g][:, n0:n0 + nl],
                                    w_sb[:, k, :],
                                    x_pair[p][k][:, t0 + n0:t0 + n0 + nl],
                                    start=(k == 0), stop=(k == DCH - 1),
                                    skip_group_check=True,
                                )
                    for g, (p, t0, tl) in enumerate(groups):
                        nc.vector.tensor_scalar_add(
                            dst[:, p, t0:t0 + tl], pst[g][:, 0:tl],
                            b_sb[:, 0:1])

            # ---- attention-phase sbuf (reuses the freed xq/xk region) ----
            bc_pools = (
                tc.tile_pool(name="vxh", bufs=1),
                tc.tile_pool(name="expp", bufs=36),
                tc.tile_pool(name="outp", bufs=4),
                tc.tile_pool(name="normp", bufs=2),
            )
            vxh_pool = bc_pools[0].__enter__()
            exp_pool = bc_pools[1].__enter__()
            out_pool = bc_pools[2].__enter__()
            norm_pool = bc_pools[3].__enter__()

            v_sb = vxh_pool.tile([128, 2, KC, 2, 66], BF16, tag="v")
            xh_sb = [vxh_pool.tile([128, 2, 1024], BF16, tag=f"xh{q}",
                                   name=f"xh{q}") for q in (0, 1)]
            nc.vector.memset(v_sb[:, :, :, :, 64:65], 1.0)

            def emit_vproj(p, i):
                ps = psp.tile([128, 128], F32, tag="ps", name="vps")
                for k in range(DCH):
                    nc.tensor.matmul(
                        ps[:],
                        xv_sb[p][k][:, i * 128:(i + 1) * 128],
                        wv_sb[:, k, :],
                        start=(k == 0), stop=(k == DCH - 1),
                        skip_group_check=True,
                    )
                nc.vector.tensor_copy(v_sb[:, p, i, :, 0:64], ps[:])

            def emit_v(prev, i):
                qh, p, o2, exs = prev
                for hh in range(2):
                    for n in range(2):
                        nc.tensor.matmul(
                            o2[hh][:, n * 512:(n + 1) * 512],
                            v_sb[:, p, i, hh, 0:65],
                            exs[i][hh][:, n * 512:(n + 1) * 512],
                            start=(i == 0), stop=(i == nkc[p] - 1),
                            skip_group_check=True,
                        )

            def emit_norm(prev, half=None):
                qh, p, o2, exs = prev
                sl = slice(0, 1024) if half is None else \
                    slice(half * 512, (half + 1) * 512)
                w = sl.stop - sl.start
                for hh in range(2):
                    rr = norm_pool.tile([1, 2, 1024], F32, tag="rr", name="rr")
                    nc.vector.tensor_copy(rr[:, 0, :w], o2[hh][64:65, sl])
                    nc.vector.reciprocal_approx_fast(rr[:, 1, :w], rr[:, 0, :w])
                    rb = norm_pool.tile([64, 1024], F32, tag="rb", name="rb")
                    nc.gpsimd.partition_broadcast(rb[:, :w], rr[:, 1, :w])
                    if hh == 0:
                        nc.vector.tensor_mul(
                            xh_sb[qh][0:64, p, sl], o2[hh][0:64, sl],
                            rb[:, :w])
                        if with_bv:
                            nc.vector.tensor_scalar_add(
                                xh_sb[qh][0:64, p, sl],
                                xh_sb[qh][0:64, p, sl],
                                bv_sb[:, 0:1])
                    else:
                        tmp = norm_pool.tile([64, 1024], BF16, tag="tmp",
                                             name="tmp")
                        nc.vector.tensor_mul(tmp[:, :w], o2[hh][0:64, sl],
                                             rb[:, :w])
                        if with_bv:
                            nc.vector.tensor_scalar_add(
                                tmp[:, :w], tmp[:, :w], bv_sb[:, 1:2])
                        nc.sync.dma_start(
                            out=xh_sb[qh][64:128, p, sl], in_=tmp[:, :w])

            def emit_outproj(qh, tr, p, tail=False):
                po = psp.tile([128, 1024], F32, tag="ps", name="po")
                for n in range(2):
                    nc.tensor.matmul(
                        po[:, n * 512:(n + 1) * 512],
                        xh_sb[qh][:, p, tr * 128:(tr + 1) * 128],
                        wo_sb[:, n * 512:(n + 1) * 512],
                        start=True, stop=True,
                        skip_group_check=True,
                    )
                ot = out_pool.tile([128, 1024], BF16, tag="ot")
                if tail and tr % 2 == 0:
                    nc.scalar.copy(ot[:], po[:])
                else:
                    nc.vector.tensor_copy(ot[:], po[:])
                t0 = qh * 1024 + tr * 128
                nc.sync.dma_start(out=out_d[p, t0:t0 + 128, :], in_=ot[:])

            def dole(njobs, nslots, j):
                """jobs assigned to slot j when njobs spread over nslots."""
                return range((njobs * j) // nslots,
                             (njobs * (j + 1)) // nslots)

            units = [(0, 0), (0, 1), (1, 0), (1, 1)]   # (q-half, pair)
            vgroups = [(0, i) for i in range(na)] + \
                      [(1, i) for i in range(nb)]
            prev = None
            for ui, (qh, p) in enumerate(units):
                n_cur = nkc[p]
                q0 = qh * 1024
                o2 = [psp.tile([65, 1024], F32, tag="ps", name="o2")
                      for _ in range(2)]
                exs = []
                oslot = 0
                for j in range(n_cur):
                    se = [psp.tile([128, 1024], F32, tag="ps", name="se")
                          for _ in range(2)]
                    if prev is not None:
                        for i in dole(nkc[prev[1]], n_cur, j):
                            emit_v(prev, i)
                    for hh in range(2):
                        pb = 64 * hh
                        for n in range(2):
                            nc.tensor.matmul(
                                se[hh][:, n * 512:(n + 1) * 512],
                                kT_sb[pb:pb + 64, p, j * 128:(j + 1) * 128],
                                qT_sb[pb:pb + 64, p, q0 + n * 512:
                                      q0 + (n + 1) * 512],
                                start=True, stop=True,
                            )
                    if ui == 0:
                        for g in dole(len(vgroups), n_cur, j):
                            emit_vproj(*vgroups[g])
                    ex = [exp_pool.tile([128, 1024], BF16, tag="ex", name="ex")
                          for _ in range(2)]
                    for hh in range(2):
                        nc.scalar.activation(
                            ex[hh][:], se[hh][:],
                            mybir.ActivationFunctionType.Exp,
                            bias=mask_sb[:, p, j:j + 1],
                            scale=float(DK) ** -0.5,
                        )
                    exs.append(ex)
                    if ui >= 2:
                        # qh=0 out-projections: pair 0 (xh normed at unit 1's
                        # end) rides unit 2; pair 1 (normed at unit 2's end)
                        # rides unit 3
                        for tr in dole(8, n_cur, j):
                            emit_outproj(0, tr, ui - 2)
                if prev is not None:
                    emit_norm(prev)
                prev = (qh, p, o2, exs)

            # tail: last unit's V accumulation in q-halves; qh=1 out-projs
            # follow each half's norm, interleaved with the other half's V.
            n_prev = nkc[prev[1]]
            for half in range(2):
                n0 = half * 512
                for i in range(n_prev):
                    for hh in range(2):
                        nc.tensor.matmul(
                            prev[2][hh][:, n0:n0 + 512],
                            v_sb[:, prev[1], i, hh, 0:65],
                            prev[3][i][hh][:, n0:n0 + 512],
                            start=(i == 0), stop=(i == n_prev - 1),
                            skip_group_check=True,
                        )
                emit_norm(prev, half=half)
                if half == 0:
                    continue
                for tr in range(4):
                    for p in range(2):
                        emit_outproj(1, tr, p, tail=True)
            for tr in range(4, 8):
                for p in range(2):
                    emit_outproj(1, tr, p, tail=True)

            for _p in reversed(bc_pools):
                _p.__exit__(None, None, None)
            xvp_cm.__exit__(None, None, None)

    nc.compile()
    return nc


_CACHE = {}


def _get_program(active_a, active_b, with_bv):
    key = (active_a, active_b, with_bv)
    if key not in _CACHE:
        _CACHE[key] = build_program(active_a, active_b, with_bv)
    return _CACHE[key]


def make_in_maps(query, key, value, mask, Wq, bq, Wk, bk, Wv, bv, Wo, bo,
                 active=((), ())):
    bf = ml_dtypes.bfloat16
    xt = {}
    for nm, x in (("xq", query), ("xk", key), ("xv", value)):
        xt[nm] = np.ascontiguousarray(
            np.stack([x[b].T.reshape(DCH, 128, T) for b in range(B)])
        ).astype(bf)
    mb = np.zeros((128, B, KC), np.float32)
    for b in range(B):
        mcols = np.where(mask[b, 0] != 0, 0.0, MASK_NEG) \
            .astype(np.float32).reshape(KC, 128).T
        # packed: column j holds the j-th ACTIVE chunk's mask bias
        for j, kc in enumerate(active[b]):
            mb[:, b, j] = mcols[:, kc]
    mb = np.ascontiguousarray(mb)
    in_maps = []
    for c in range(NCORES):
        cols = slice(128 * c, 128 * (c + 1))
        m = {"xq": xt["xq"], "xk": xt["xk"], "xv": xt["xv"], "maskb": mb}
        for nm, W in (("wq", Wq), ("wk", Wk), ("wv", Wv)):
            m[nm] = np.ascontiguousarray(
                W[cols, :].T.reshape(DCH, 128, 128).transpose(1, 0, 2)
            ).astype(bf)
        m["wo"] = np.ascontiguousarray(Wo[:, cols].T).astype(bf)
        m["bq"] = np.ascontiguousarray(
            bq[cols].reshape(128, 1).astype(np.float32))
        m["bk"] = np.ascontiguousarray(
            bk[cols].reshape(128, 1).astype(np.float32))
        m["bv"] = np.ascontiguousarray(
            bv[cols].reshape(2, 64).T.astype(np.float32))
        in_maps.append(m)
    return in_maps


def kernel(query, key, value, mask, Wq, bq, Wk, bk, Wv, bv, Wo, bo,
           _trace=False):
    query, key, value = (np.asarray(a, np.float32) for a in (query, key, value))
    mask = np.asarray(mask)
    with_bv = bool(np.any(np.asarray(bv)))
    active = []
    for b in range(B):
        mb = np.asarray(mask[b, 0]) != 0
        active.append(tuple(
            kc for kc in range(KC) if mb[kc * 128:(kc + 1) * 128].any()))
    nc = _get_program(active[0], active[1], with_bv)
    in_maps = make_in_maps(query, key, value, mask, Wq, bq, Wk, bk, Wv, bv,
                           Wo, bo, active=active)
    res = run_bass_kernel_spmd(nc, in_maps, list(range(NCORES)), trace=_trace)
    out = np.zeros((B, T, D), np.float32)
    for c in range(NCORES):
        out += res.results[c]["out"].astype(np.float32)
    out += np.asarray(bo, np.float32)[None, None, :]
    if _trace:
        kernel.last_exec_time_ns = res.exec_time_ns
        kernel.last_results = res
    return out
